# revision 21
# baseline (speedup 1.0000x reference)
"""AdaptiveNormalization Trainium2 kernel (8 NeuronCores, batch-parallel).

Reference computation (per batch b):
    a      = ema(x, m)                      # causal EMA over T, per (b,c)
    shift  = sum_c w_shift[c] * a[c,t]      # (b,t)
    x1     = x - shift
    bb     = ema(x1^2, m)
    scale  = sum_c exp(w_scale_log)[c] * bb[c,t]
    out    = (x1 / sqrt(scale+eps)) * w_proj[c] + b_proj[c]

Rewrites used here:
  * The EMA is linear and channel-independent, so the channel reduction
    commutes with it:  shift = ema(s) with s_t = sum_c w_shift[c] x[c,t],
    and scale = ema(q) with q_t = u_t - 2 shift_t v_t + shift_t^2 E,
    u = sum e_c x^2, v = sum e_c x, E = sum e_c.
  * w_proj is folded into x on the host (xt = w_proj * x, bf16), so
    out = xt*inv - (w_proj*si - b_proj)  with  si = shift*inv,
    inv = 1/sqrt(scale+eps).  Stat weights are divided by w_proj (w_proj^2
    for the square stat) to compensate.
  * All I/O is bf16 (tolerance is 2e-2); stats/scans run in f32.

Implementation notes:
  * Stats are bf16 PE matmuls quadrant-packed at PSUM base partitions
    {0,32,64} so one ACT copy drains 3 groups (engine cost scales with
    free size only).
  * The per-(32-block) scan carries are computed with small PE matmuls
    (lower-triangular r^128-power matrix), avoiding per-scan DMAs.
  * inv/si rows are replicated to 128 partitions with a 7-step SBUF->SBUF
    DMA doubling cascade, keeping the phase-3 DVE ops all-bf16-SBUF
    (tensor_tensor 2x mode, tensor_scalar 4x mode).
"""

import sys
import os

for _p in ("/opt/trn_rl_repo",):
    if _p not in sys.path:
        sys.path.insert(0, _p)

import numpy as np
import ml_dtypes
from contextlib import ExitStack

import concourse.bass as bass
import concourse.bacc as bacc
import concourse.tile as tile
from concourse import mybir
from concourse import bass_utils

MOMENTUM = 0.01
EPS = 1e-6
B, C, T_FULL = 8, 256, 16384
N_CORES = 8
BS = 128          # scan block size (columns per scan block)
NSEG = 8          # pipeline segments over T

F32 = mybir.dt.float32
BF16 = mybir.dt.bfloat16
AOP = mybir.AluOpType
ACTF = mybir.ActivationFunctionType
BF = ml_dtypes.bfloat16


def _host_constants(w_shift, w_scale_log, w_proj, b_proj, T):
    m = MOMENTUM
    r = 1.0 - m
    SEG = T // NSEG
    NB = SEG // BS
    r128 = r ** BS

    ws = w_shift.astype(np.float64)
    e = np.exp(w_scale_log.astype(np.float64))
    wp = w_proj.astype(np.float64)
    bp = b_proj.astype(np.float64)

    # Stat weights, folded for xt = wp*x, zero-padded to 32 output columns
    # (so each quadrant matmul initializes a full 32-partition stripe).
    # The u matmul accumulates into the same PSUM stripe as s/v with its
    # stat at column 2, so one stripe ends up holding [s, v, u, 0...].
    w_sv = np.zeros((128, 2, 32), np.float64)
    w_u = np.zeros((128, 2, 32), np.float64)
    for h in (0, 1):
        sl = slice(128 * h, 128 * (h + 1))
        w_sv[:, h, 0] = m * ws[sl] / wp[sl]
        w_sv[:, h, 1] = -2.0 * m * e[sl] / wp[sl]
        w_u[:, h, 2] = m * e[sl] / wp[sl] ** 2

    # per-half per-partition scalars for the D tensor_scalar
    wpb = np.zeros((128, 2, 2), np.float64)
    for h in (0, 1):
        sl = slice(128 * h, 128 * (h + 1))
        wpb[:, h, 0] = wp[sl]
        wpb[:, h, 1] = bp[sl]

    ecolm = np.full((NB, 1), m * e.sum(), np.float64)

    # scan-carry matmul constants
    mcarry = np.zeros((NB, NB), np.float64)   # lhsT[k, j] = r128^(j-1-k), k<=j-1
    for j in range(NB):
        for k in range(j):
            mcarry[k, j] = r128 ** (j - 1 - k)
    rpow = np.zeros((1, NB), np.float64)
    rpow[0, :] = r128 ** np.arange(NB)
    elast = np.zeros((NB, 1), np.float64)
    elast[NB - 1, 0] = 1.0

    f = lambda a: np.ascontiguousarray(a, dtype=np.float32)
    bf = lambda a: np.ascontiguousarray(a.astype(np.float32), dtype=BF)
    return dict(
        w_sv=bf(w_sv), w_u=bf(w_u), wpb=f(wpb), e_col=f(ecolm),
        mcarry=f(mcarry), rpow=f(rpow), elast=f(elast),
    )


def build_model(T=T_FULL):
    m = MOMENTUM
    r = 1.0 - m
    SEG = T // NSEG
    NB = SEG // BS
    NGS = SEG // 512          # 512-column stat groups per segment

    nc = bacc.Bacc("TRN2", target_bir_lowering=False, debug=False)

    x_d = nc.dram_tensor("x", [128, 2, T], BF16, kind="ExternalInput")
    wsv_d = nc.dram_tensor("w_sv", [128, 2, 32], BF16, kind="ExternalInput")
    wu_d = nc.dram_tensor("w_u", [128, 2, 32], BF16, kind="ExternalInput")
    wpb_d = nc.dram_tensor("wpb", [128, 2, 2], F32, kind="ExternalInput")
    ecol_d = nc.dram_tensor("e_col", [NB, 1], F32, kind="ExternalInput")
    mcar_d = nc.dram_tensor("mcarry", [NB, NB], F32, kind="ExternalInput")
    rpow_d = nc.dram_tensor("rpow", [1, NB], F32, kind="ExternalInput")
    elast_d = nc.dram_tensor("elast", [NB, 1], F32, kind="ExternalInput")
    out_d = nc.dram_tensor("out", [128, 2, T], BF16, kind="ExternalOutput")

    with tile.TileContext(nc) as tc, ExitStack() as ctx:
        consts = ctx.enter_context(tc.tile_pool(name="consts", bufs=1))
        xpool = ctx.enter_context(tc.tile_pool(name="x", bufs=4))
        sqpool = ctx.enter_context(tc.tile_pool(name="sq", bufs=2))
        stpool = ctx.enter_context(tc.tile_pool(name="stage", bufs=2))
        scanp = ctx.enter_context(tc.tile_pool(name="scan", bufs=2))
        bcpool = ctx.enter_context(tc.tile_pool(name="bc", bufs=4))
        zpool = ctx.enter_context(tc.tile_pool(name="z", bufs=2))
        dpool = ctx.enter_context(tc.tile_pool(name="d", bufs=2))
        opool = ctx.enter_context(tc.tile_pool(name="o", bufs=2))
        ps_stat = ctx.enter_context(tc.tile_pool(name="pstat", bufs=2,
                                                 space="PSUM"))
        ps_car = ctx.enter_context(tc.tile_pool(name="pcar", bufs=2,
                                                space="PSUM"))

        # ---- constants ----
        wsv_sb = consts.tile([128, 2, 32], BF16)
        wu_sb = consts.tile([128, 2, 32], BF16)
        wpb_sb = consts.tile([128, 2, 2], F32)
        ecol_sb = consts.tile([NB, 1], F32)
        mcar_sb = consts.tile([NB, NB], F32)
        rpow_sb = consts.tile([1, NB], F32)
        elast_sb = consts.tile([NB, 1], F32)
        for sb, d in ((wsv_sb, wsv_d), (wu_sb, wu_d), (wpb_sb, wpb_d),
                      (ecol_sb, ecol_d), (mcar_sb, mcar_d),
                      (rpow_sb, rpow_d), (elast_sb, elast_d)):
            nc.sync.dma_start(sb[:], d[:])
        rfill = consts.tile([NB, BS], F32)
        nc.vector.memset(rfill[:], r)
        eps_sb = consts.tile([NB, 1], F32)
        nc.vector.memset(eps_sb[:], EPS)
        cin = {"shift": consts.tile([1, 1], F32, name="cin_shift"),
               "scale": consts.tile([1, 1], F32, name="cin_scale")}
        nc.vector.memset(cin["shift"][:], 0.0)
        nc.vector.memset(cin["scale"][:], 0.0)

        def ema_scan(src, sig, ccol4, ccol_i, cine_i):
            """Blockwise EMA scan of src [NB, BS] with PE-matmul carries.
            Chains across segments through cin[sig]."""
            loc = scanp.tile([NB, BS], F32, tag=f"{sig}_loc")
            nc.vector.tensor_tensor_scan(
                loc[:], rfill[:], src[:], 0.0, AOP.mult, AOP.add)
            # carry into block j: P_j = sum_{k<j} r128^{j-1-k} y_k + r128^j cin
            nc.tensor.matmul(ccol4[0:NB, ccol_i:ccol_i + 1],
                             lhsT=mcar_sb[:], rhs=loc[:, BS - 1:BS],
                             start=True, stop=False)
            nc.tensor.matmul(ccol4[0:NB, ccol_i:ccol_i + 1],
                             lhsT=rpow_sb[:], rhs=cin[sig][:],
                             start=False, stop=True)
            fixed = scanp.tile([NB, BS], F32, tag=f"{sig}_fix")
            nc.vector.tensor_tensor_scan(
                fixed[:], rfill[:], src[:], ccol4[0:NB, ccol_i:ccol_i + 1],
                AOP.mult, AOP.add)
            # next segment's carry-in = last value of this segment
            nc.tensor.matmul(ccol4[0:1, cine_i:cine_i + 1],
                             lhsT=elast_sb[:], rhs=fixed[:, BS - 1:BS],
                             start=True, stop=True)
            nc.vector.tensor_copy(cin[sig][:], ccol4[0:1, cine_i:cine_i + 1])
            return fixed

        GW = 1024                 # stat group width (columns)
        NG = SEG // GW            # stat groups per segment
        NBG = GW // BS            # scan blocks per group

        for s in range(NSEG):
            seg0 = s * SEG

            # ---- x load (one DMA per segment, sync queue = big I/O only) ----
            xs = xpool.tile([128, 2, SEG], BF16, tag="x")
            nc.sync.dma_start(xs[:], x_d[:, :, seg0:seg0 + SEG])

            # ---- phase 1: stats; groups of GW cols, 3 per PSUM tile at
            # quadrant bases {0,32,64}; u accumulates into the same stripe ----
            S_s = scanp.tile([NB, BS], F32, tag="S_s")
            S_v = scanp.tile([NB, BS], F32, tag="S_v")
            S_u = scanp.tile([NB, BS], F32, tag="S_u")
            g0 = 0
            while g0 < NG:
                ng = min(3, NG - g0)
                pp = 32 * ng
                svu_ps = ps_stat.tile([96, GW], F32, tag="svu")
                for gi in range(ng):
                    g = g0 + gi
                    cols = slice(GW * g, GW * (g + 1))
                    qb = 32 * gi
                    sq = sqpool.tile([128, 2, GW], BF16, tag="sq")
                    for h in (0, 1):
                        nc.scalar.activation(sq[:, h, :], xs[:, h, cols],
                                             ACTF.Square)
                    for j in (0, 1):
                        jsl = slice(512 * j, 512 * (j + 1))
                        jc = slice(cols.start + 512 * j,
                                   cols.start + 512 * (j + 1))
                        nc.tensor.matmul(svu_ps[qb:qb + 32, jsl],
                                         lhsT=wsv_sb[:, 0, :],
                                         rhs=xs[:, 0, jc],
                                         start=True, stop=False)
                        nc.tensor.matmul(svu_ps[qb:qb + 32, jsl],
                                         lhsT=wsv_sb[:, 1, :],
                                         rhs=xs[:, 1, jc],
                                         start=False, stop=False)
                        nc.tensor.matmul(svu_ps[qb:qb + 32, jsl],
                                         lhsT=wu_sb[:, 0, :],
                                         rhs=sq[:, 0, jsl],
                                         start=False, stop=False)
                        nc.tensor.matmul(svu_ps[qb:qb + 32, jsl],
                                         lhsT=wu_sb[:, 1, :],
                                         rhs=sq[:, 1, jsl],
                                         start=False, stop=True)
                stage = stpool.tile([96, GW], F32, tag="stg")
                nc.scalar.copy(stage[0:pp, :], svu_ps[0:pp, :])
                # scatter stat rows to scan-block layout (gpsimd SWDGE queue)
                for gi in range(ng):
                    g = g0 + gi
                    bsl = slice(NBG * g, NBG * (g + 1))
                    nc.gpsimd.dma_start(S_s[bsl, :],
                                        stage[32 * gi + 0:32 * gi + 1, :])
                    nc.gpsimd.dma_start(S_v[bsl, :],
                                        stage[32 * gi + 1:32 * gi + 2, :])
                    nc.gpsimd.dma_start(S_u[bsl, :],
                                        stage[32 * gi + 2:32 * gi + 3, :])
                g0 += ng

            # ---- phase 2: scans ----
            ccol4 = ps_car.tile([NB, 4], F32, tag="ccol")
            shift_S = ema_scan(S_s, "shift", ccol4, 0, 2)

            t1 = scanp.tile([NB, BS], F32, tag="t1")
            nc.vector.tensor_tensor(t1[:], shift_S[:], S_v[:], AOP.mult)
            t2 = scanp.tile([NB, BS], F32, tag="t2")
            nc.vector.tensor_tensor(t2[:], shift_S[:], shift_S[:], AOP.mult)
            q1 = scanp.tile([NB, BS], F32, tag="q1")
            nc.vector.scalar_tensor_tensor(
                q1[:], t2[:], ecol_sb[:], S_u[:], AOP.mult, AOP.add)
            qm = scanp.tile([NB, BS], F32, tag="qm")
            nc.vector.tensor_tensor(qm[:], q1[:], t1[:], AOP.add)

            scale_S = ema_scan(qm, "scale", ccol4, 1, 3)

            sq_s = scanp.tile([NB, BS], F32, tag="sq_s")
            nc.scalar.activation(sq_s[:], scale_S[:], ACTF.Sqrt,
                                 bias=eps_sb[:])
            inv_S = scanp.tile([NB, BS], F32, tag="inv_S")
            nc.vector.reciprocal_approx_fast(inv_S[:], sq_s[:])
            sib = scanp.tile([NB, BS], BF16, tag="sib")
            nc.vector.tensor_tensor(sib[:], shift_S[:], inv_S[:], AOP.mult)
            invb = scanp.tile([NB, BS], BF16, tag="invb")
            nc.vector.tensor_copy(invb[:], inv_S[:])

            # ---- broadcast rows to 128 partitions (doubling cascade) ----
            bc = bcpool.tile([128, 2 * SEG], BF16, tag="bc")
            nc.gpsimd.dma_start(bc[0:1, 0:SEG], invb[:])
            nc.gpsimd.dma_start(bc[0:1, SEG:2 * SEG], sib[:])
            # doubling cascade to 16 partitions, then an independent fan-out
            # (the 7 fan DMAs have no mutual deps -> they pipeline).
            # All on the sync queue: the scalar queue shares the ACT
            # sequencer and would stall squares/copies/sqrt.
            k = 1
            while k < 16:
                nc.sync.dma_start(bc[k:2 * k, :], bc[0:k, :])
                k *= 2
            for j in range(1, 8):
                nc.sync.dma_start(bc[16 * j:16 * (j + 1), :], bc[0:16, :])

            # ---- phase 3: out = xt*inv - (wp*si - bp) ----
            bcinv = bass.AP(bc[:, 0:SEG].tensor, bc[:, 0:SEG].offset,
                            [list(bc[:, 0:SEG].ap[0]), [0, 2],
                             list(bc[:, 0:SEG].ap[1])])
            z = zpool.tile([128, 2, SEG], BF16, tag="z")
            nc.vector.tensor_tensor(z[:], xs[:], bcinv, AOP.mult)
            o = opool.tile([128, 2, SEG], BF16, tag="o")
            for h in (0, 1):
                D = dpool.tile([128, SEG], BF16, tag="D")
                nc.vector.tensor_scalar(D[:], bc[:, SEG:2 * SEG],
                                        wpb_sb[:, h, 0:1], wpb_sb[:, h, 1:2],
                                        AOP.mult, AOP.subtract)
                nc.vector.tensor_tensor(o[:, h, :], z[:, h, :], D[:],
                                        AOP.subtract)
            nc.sync.dma_start(out_d[:, :, seg0:seg0 + SEG], o[:])

    nc.compile()
    return nc


_MODEL_CACHE = {}


def _get_model(T=T_FULL):
    if T not in _MODEL_CACHE:
        _MODEL_CACHE[T] = build_model(T)
    return _MODEL_CACHE[T]


def make_in_maps(x, w_shift, w_scale_log, w_proj, b_proj, T):
    """Per-core input dicts (core i gets batch i)."""
    consts = _host_constants(w_shift, w_scale_log, w_proj, b_proj, T)
    nb = x.shape[0]
    wp = w_proj.astype(np.float32)
    in_maps = []
    for i in range(nb):
        xt = (x[i].astype(np.float32) * wp[:, None]).astype(BF)
        xt = np.ascontiguousarray(
            np.stack([xt[:128], xt[128:]], axis=1))      # [128, 2, T]
        im = {"x": xt}
        im.update(consts)
        in_maps.append(im)
    return in_maps


def kernel(x, w_shift, w_scale_log, w_proj, b_proj):
    T = x.shape[-1]
    nc = _get_model(T)
    in_maps = make_in_maps(x, w_shift, w_scale_log, w_proj, b_proj, T)
    res = bass_utils.run_bass_kernel_spmd(
        nc, in_maps, core_ids=list(range(len(in_maps))))
    outs = []
    for i in range(len(in_maps)):
        o = np.asarray(res.results[i]["out"])            # [128, 2, T] bf16
        outs.append(np.concatenate([o[:, 0, :], o[:, 1, :]], axis=0))
    return np.stack(outs, 0).astype(np.float32)


# revision 22
# speedup vs baseline: 1.2105x; 1.2105x over previous
"""AdaptiveNormalization Trainium2 kernel (8 NeuronCores, batch-parallel).

Reference computation (per batch b):
    a      = ema(x, m)                      # causal EMA over T, per (b,c)
    shift  = sum_c w_shift[c] * a[c,t]      # (b,t)
    x1     = x - shift
    bb     = ema(x1^2, m)
    scale  = sum_c exp(w_scale_log)[c] * bb[c,t]
    out    = (x1 / sqrt(scale+eps)) * w_proj[c] + b_proj[c]

Rewrites used here:
  * The EMA is linear and channel-independent, so the channel reduction
    commutes with it:  shift = ema(s) with s_t = sum_c w_shift[c] x[c,t],
    and scale = ema(q) with q_t = u_t - 2 shift_t v_t + shift_t^2 E,
    u = sum e_c x^2, v = sum e_c x, E = sum e_c.
  * w_proj is folded into x on the host (xt = w_proj * x, bf16), so
    out = xt*inv - (w_proj*si - b_proj)  with  si = shift*inv,
    inv = 1/sqrt(scale+eps).  Stat weights are divided by w_proj (w_proj^2
    for the square stat) to compensate.
  * All I/O is bf16 (tolerance is 2e-2); stats/scans run in f32.

Implementation notes:
  * Stats are bf16 PE matmuls quadrant-packed at PSUM base partitions
    {0,32,64} so one ACT copy drains 3 groups (engine cost scales with
    free size only).
  * The per-(32-block) scan carries are computed with small PE matmuls
    (lower-triangular r^128-power matrix), avoiding per-scan DMAs.
  * inv/si rows are replicated to 128 partitions with a 7-step SBUF->SBUF
    DMA doubling cascade, keeping the phase-3 DVE ops all-bf16-SBUF
    (tensor_tensor 2x mode, tensor_scalar 4x mode).
"""

import sys
import os

for _p in ("/opt/trn_rl_repo",):
    if _p not in sys.path:
        sys.path.insert(0, _p)

import numpy as np
import ml_dtypes
from contextlib import ExitStack

import concourse.bass as bass
import concourse.bacc as bacc
import concourse.tile as tile
from concourse import mybir
from concourse import bass_utils

MOMENTUM = 0.01
EPS = 1e-6
B, C, T_FULL = 8, 256, 16384
N_CORES = 8
BS = 128          # scan block size (columns per scan block)
NSEG = 4          # pipeline segments over T

F32 = mybir.dt.float32
BF16 = mybir.dt.bfloat16
AOP = mybir.AluOpType
ACTF = mybir.ActivationFunctionType
BF = ml_dtypes.bfloat16


def _host_constants(w_shift, w_scale_log, w_proj, b_proj, T):
    m = MOMENTUM
    r = 1.0 - m
    SEG = T // NSEG
    NB = SEG // BS
    r128 = r ** BS

    ws = w_shift.astype(np.float64)
    e = np.exp(w_scale_log.astype(np.float64))
    wp = w_proj.astype(np.float64)
    bp = b_proj.astype(np.float64)

    # Stat weights, folded for xt = wp*x, zero-padded to 32 output columns
    # (so each quadrant matmul initializes a full 32-partition stripe).
    # The u matmul accumulates into the same PSUM stripe as s/v with its
    # stat at column 2, so one stripe ends up holding [s, v, u, 0...].
    w_sv = np.zeros((128, 2, 32), np.float64)
    w_u = np.zeros((128, 2, 32), np.float64)
    for h in (0, 1):
        sl = slice(128 * h, 128 * (h + 1))
        w_sv[:, h, 0] = m * ws[sl] / wp[sl]
        w_sv[:, h, 1] = -2.0 * m * e[sl] / wp[sl]
        w_u[:, h, 2] = m * e[sl] / wp[sl] ** 2

    # per-half per-partition scalars for the D tensor_scalar
    wpb = np.zeros((128, 2, 2), np.float64)
    for h in (0, 1):
        sl = slice(128 * h, 128 * (h + 1))
        wpb[:, h, 0] = wp[sl]
        wpb[:, h, 1] = bp[sl]

    ecolm = np.full((NB, 1), m * e.sum(), np.float64)

    # scan-carry matmul constants
    mcarry = np.zeros((NB, NB), np.float64)   # lhsT[k, j] = r128^(j-1-k), k<=j-1
    for j in range(NB):
        for k in range(j):
            mcarry[k, j] = r128 ** (j - 1 - k)
    rpow = np.zeros((1, NB), np.float64)
    rpow[0, :] = r128 ** np.arange(NB)
    elast = np.zeros((NB, 1), np.float64)
    elast[NB - 1, 0] = 1.0

    f = lambda a: np.ascontiguousarray(a, dtype=np.float32)
    bf = lambda a: np.ascontiguousarray(a.astype(np.float32), dtype=BF)
    return dict(
        w_sv=bf(w_sv), w_u=bf(w_u), wpb=f(wpb), e_col=f(ecolm),
        mcarry=f(mcarry), rpow=f(rpow), elast=f(elast),
    )


def build_model(T=T_FULL):
    m = MOMENTUM
    r = 1.0 - m
    SEG = T // NSEG
    NB = SEG // BS
    NGS = SEG // 512          # 512-column stat groups per segment

    nc = bacc.Bacc("TRN2", target_bir_lowering=False, debug=False)

    x_d = nc.dram_tensor("x", [128, 2, T], BF16, kind="ExternalInput")
    wsv_d = nc.dram_tensor("w_sv", [128, 2, 32], BF16, kind="ExternalInput")
    wu_d = nc.dram_tensor("w_u", [128, 2, 32], BF16, kind="ExternalInput")
    wpb_d = nc.dram_tensor("wpb", [128, 2, 2], F32, kind="ExternalInput")
    ecol_d = nc.dram_tensor("e_col", [NB, 1], F32, kind="ExternalInput")
    mcar_d = nc.dram_tensor("mcarry", [NB, NB], F32, kind="ExternalInput")
    rpow_d = nc.dram_tensor("rpow", [1, NB], F32, kind="ExternalInput")
    elast_d = nc.dram_tensor("elast", [NB, 1], F32, kind="ExternalInput")
    out_d = nc.dram_tensor("out", [128, 2, T], BF16, kind="ExternalOutput")

    with tile.TileContext(nc) as tc, ExitStack() as ctx:
        consts = ctx.enter_context(tc.tile_pool(name="consts", bufs=1))
        xpool = ctx.enter_context(tc.tile_pool(name="x", bufs=2))
        sqpool = ctx.enter_context(tc.tile_pool(name="sq", bufs=2))
        stpool = ctx.enter_context(tc.tile_pool(name="stage", bufs=2))
        scanp = ctx.enter_context(tc.tile_pool(name="scan", bufs=2))
        bcpool = ctx.enter_context(tc.tile_pool(name="bc", bufs=2))
        zpool = ctx.enter_context(tc.tile_pool(name="z", bufs=2))
        dpool = ctx.enter_context(tc.tile_pool(name="d", bufs=2))
        opool = ctx.enter_context(tc.tile_pool(name="o", bufs=2))
        ps_stat = ctx.enter_context(tc.tile_pool(name="pstat", bufs=2,
                                                 space="PSUM"))
        ps_car = ctx.enter_context(tc.tile_pool(name="pcar", bufs=2,
                                                space="PSUM"))

        # ---- constants ----
        wsv_sb = consts.tile([128, 2, 32], BF16)
        wu_sb = consts.tile([128, 2, 32], BF16)
        wpb_sb = consts.tile([128, 2, 2], F32)
        ecol_sb = consts.tile([NB, 1], F32)
        mcar_sb = consts.tile([NB, NB], F32)
        rpow_sb = consts.tile([1, NB], F32)
        elast_sb = consts.tile([NB, 1], F32)
        for sb, d in ((wsv_sb, wsv_d), (wu_sb, wu_d), (wpb_sb, wpb_d),
                      (ecol_sb, ecol_d), (mcar_sb, mcar_d),
                      (rpow_sb, rpow_d), (elast_sb, elast_d)):
            nc.sync.dma_start(sb[:], d[:])
        rfill = consts.tile([NB, BS], F32)
        nc.vector.memset(rfill[:], r)
        eps_sb = consts.tile([NB, 1], F32)
        nc.vector.memset(eps_sb[:], EPS)
        cin = {"shift": consts.tile([1, 1], F32, name="cin_shift"),
               "scale": consts.tile([1, 1], F32, name="cin_scale")}
        nc.vector.memset(cin["shift"][:], 0.0)
        nc.vector.memset(cin["scale"][:], 0.0)

        def ema_scan(src, sig, ccol4, ccol_i, cine_i):
            """Blockwise EMA scan of src [NB, BS] with PE-matmul carries.
            Chains across segments through cin[sig]."""
            loc = scanp.tile([NB, BS], F32, tag=f"{sig}_loc")
            nc.vector.tensor_tensor_scan(
                loc[:], rfill[:], src[:], 0.0, AOP.mult, AOP.add)
            # carry into block j: P_j = sum_{k<j} r128^{j-1-k} y_k + r128^j cin
            nc.tensor.matmul(ccol4[0:NB, ccol_i:ccol_i + 1],
                             lhsT=mcar_sb[:], rhs=loc[:, BS - 1:BS],
                             start=True, stop=False)
            nc.tensor.matmul(ccol4[0:NB, ccol_i:ccol_i + 1],
                             lhsT=rpow_sb[:], rhs=cin[sig][:],
                             start=False, stop=True)
            fixed = scanp.tile([NB, BS], F32, tag=f"{sig}_fix")
            nc.vector.tensor_tensor_scan(
                fixed[:], rfill[:], src[:], ccol4[0:NB, ccol_i:ccol_i + 1],
                AOP.mult, AOP.add)
            # next segment's carry-in = last value of this segment
            nc.tensor.matmul(ccol4[0:1, cine_i:cine_i + 1],
                             lhsT=elast_sb[:], rhs=fixed[:, BS - 1:BS],
                             start=True, stop=True)
            nc.vector.tensor_copy(cin[sig][:], ccol4[0:1, cine_i:cine_i + 1])
            return fixed

        GW = 1024                 # stat group width (columns)
        NG = SEG // GW            # stat groups per segment
        NBG = GW // BS            # scan blocks per group

        for s in range(NSEG):
            seg0 = s * SEG

            # ---- x load (one DMA per segment, sync queue = big I/O only) ----
            xs = xpool.tile([128, 2, SEG], BF16, tag="x")
            nc.sync.dma_start(xs[:], x_d[:, :, seg0:seg0 + SEG])

            # ---- phase 1: stats; groups of GW cols, 3 per PSUM tile at
            # quadrant bases {0,32,64}; u accumulates into the same stripe ----
            S_s = scanp.tile([NB, BS], F32, tag="S_s")
            S_v = scanp.tile([NB, BS], F32, tag="S_v")
            S_u = scanp.tile([NB, BS], F32, tag="S_u")
            g0 = 0
            while g0 < NG:
                ng = min(3, NG - g0)
                pp = 32 * ng
                svu_ps = ps_stat.tile([96, GW], F32, tag="svu")
                for gi in range(ng):
                    g = g0 + gi
                    cols = slice(GW * g, GW * (g + 1))
                    qb = 32 * gi
                    sq = sqpool.tile([128, 2, GW], BF16, tag="sq")
                    for h in (0, 1):
                        nc.scalar.activation(sq[:, h, :], xs[:, h, cols],
                                             ACTF.Square)
                    for j in (0, 1):
                        jsl = slice(512 * j, 512 * (j + 1))
                        jc = slice(cols.start + 512 * j,
                                   cols.start + 512 * (j + 1))
                        nc.tensor.matmul(svu_ps[qb:qb + 32, jsl],
                                         lhsT=wsv_sb[:, 0, :],
                                         rhs=xs[:, 0, jc],
                                         start=True, stop=False)
                        nc.tensor.matmul(svu_ps[qb:qb + 32, jsl],
                                         lhsT=wsv_sb[:, 1, :],
                                         rhs=xs[:, 1, jc],
                                         start=False, stop=False)
                        nc.tensor.matmul(svu_ps[qb:qb + 32, jsl],
                                         lhsT=wu_sb[:, 0, :],
                                         rhs=sq[:, 0, jsl],
                                         start=False, stop=False)
                        nc.tensor.matmul(svu_ps[qb:qb + 32, jsl],
                                         lhsT=wu_sb[:, 1, :],
                                         rhs=sq[:, 1, jsl],
                                         start=False, stop=True)
                stage = stpool.tile([96, GW], F32, tag="stg")
                nc.scalar.copy(stage[0:pp, :], svu_ps[0:pp, :])
                # scatter stat rows to scan-block layout (gpsimd SWDGE queue)
                for gi in range(ng):
                    g = g0 + gi
                    bsl = slice(NBG * g, NBG * (g + 1))
                    nc.gpsimd.dma_start(S_s[bsl, :],
                                        stage[32 * gi + 0:32 * gi + 1, :])
                    nc.gpsimd.dma_start(S_v[bsl, :],
                                        stage[32 * gi + 1:32 * gi + 2, :])
                    nc.gpsimd.dma_start(S_u[bsl, :],
                                        stage[32 * gi + 2:32 * gi + 3, :])
                g0 += ng

            # ---- phase 2: scans ----
            ccol4 = ps_car.tile([NB, 4], F32, tag="ccol")
            shift_S = ema_scan(S_s, "shift", ccol4, 0, 2)

            t1 = scanp.tile([NB, BS], F32, tag="t1")
            nc.vector.tensor_tensor(t1[:], shift_S[:], S_v[:], AOP.mult)
            t2 = scanp.tile([NB, BS], F32, tag="t2")
            nc.vector.tensor_tensor(t2[:], shift_S[:], shift_S[:], AOP.mult)
            q1 = scanp.tile([NB, BS], F32, tag="q1")
            nc.vector.scalar_tensor_tensor(
                q1[:], t2[:], ecol_sb[:], S_u[:], AOP.mult, AOP.add)
            qm = scanp.tile([NB, BS], F32, tag="qm")
            nc.vector.tensor_tensor(qm[:], q1[:], t1[:], AOP.add)

            scale_S = ema_scan(qm, "scale", ccol4, 1, 3)

            sq_s = scanp.tile([NB, BS], F32, tag="sq_s")
            nc.scalar.activation(sq_s[:], scale_S[:], ACTF.Sqrt,
                                 bias=eps_sb[:])
            inv_S = scanp.tile([NB, BS], F32, tag="inv_S")
            nc.vector.reciprocal_approx_fast(inv_S[:], sq_s[:])
            sib = scanp.tile([NB, BS], BF16, tag="sib")
            nc.vector.tensor_tensor(sib[:], shift_S[:], inv_S[:], AOP.mult)
            invb = scanp.tile([NB, BS], BF16, tag="invb")
            nc.vector.tensor_copy(invb[:], inv_S[:])

            # ---- broadcast rows to 128 partitions (doubling cascade) ----
            bc = bcpool.tile([128, 2 * SEG], BF16, tag="bc")
            nc.scalar.dma_start(bc[0:1, 0:SEG], invb[:])
            nc.scalar.dma_start(bc[0:1, SEG:2 * SEG], sib[:])
            k = 1
            while k < 128:
                nc.scalar.dma_start(bc[k:2 * k, :], bc[0:k, :])
                k *= 2

            # ---- phase 3: out = xt*inv - (wp*si - bp) ----
            bcinv = bass.AP(bc[:, 0:SEG].tensor, bc[:, 0:SEG].offset,
                            [list(bc[:, 0:SEG].ap[0]), [0, 2],
                             list(bc[:, 0:SEG].ap[1])])
            z = zpool.tile([128, 2, SEG], BF16, tag="z")
            nc.vector.tensor_tensor(z[:], xs[:], bcinv, AOP.mult)
            o = opool.tile([128, 2, SEG], BF16, tag="o")
            for h in (0, 1):
                D = dpool.tile([128, SEG], BF16, tag="D")
                nc.vector.tensor_scalar(D[:], bc[:, SEG:2 * SEG],
                                        wpb_sb[:, h, 0:1], wpb_sb[:, h, 1:2],
                                        AOP.mult, AOP.subtract)
                nc.vector.tensor_tensor(o[:, h, :], z[:, h, :], D[:],
                                        AOP.subtract)
            nc.sync.dma_start(out_d[:, :, seg0:seg0 + SEG], o[:])

    nc.compile()
    return nc


_MODEL_CACHE = {}


def _get_model(T=T_FULL):
    if T not in _MODEL_CACHE:
        _MODEL_CACHE[T] = build_model(T)
    return _MODEL_CACHE[T]


def make_in_maps(x, w_shift, w_scale_log, w_proj, b_proj, T):
    """Per-core input dicts (core i gets batch i)."""
    consts = _host_constants(w_shift, w_scale_log, w_proj, b_proj, T)
    nb = x.shape[0]
    wp = w_proj.astype(np.float32)
    in_maps = []
    for i in range(nb):
        xt = (x[i].astype(np.float32) * wp[:, None]).astype(BF)
        xt = np.ascontiguousarray(
            np.stack([xt[:128], xt[128:]], axis=1))      # [128, 2, T]
        im = {"x": xt}
        im.update(consts)
        in_maps.append(im)
    return in_maps


def kernel(x, w_shift, w_scale_log, w_proj, b_proj):
    T = x.shape[-1]
    nc = _get_model(T)
    in_maps = make_in_maps(x, w_shift, w_scale_log, w_proj, b_proj, T)
    res = bass_utils.run_bass_kernel_spmd(
        nc, in_maps, core_ids=list(range(len(in_maps))))
    outs = []
    for i in range(len(in_maps)):
        o = np.asarray(res.results[i]["out"])            # [128, 2, T] bf16
        outs.append(np.concatenate([o[:, 0, :], o[:, 1, :]], axis=0))
    return np.stack(outs, 0).astype(np.float32)


# revision 29
# speedup vs baseline: 1.3458x; 1.1118x over previous
"""AdaptiveNormalization Trainium2 kernel (8 NeuronCores, batch-parallel).

Reference computation (per batch b):
    a      = ema(x, m)                      # causal EMA over T, per (b,c)
    shift  = sum_c w_shift[c] * a[c,t]      # (b,t)
    x1     = x - shift
    bb     = ema(x1^2, m)
    scale  = sum_c exp(w_scale_log)[c] * bb[c,t]
    out    = (x1 / sqrt(scale+eps)) * w_proj[c] + b_proj[c]

Rewrites used here:
  * The EMA is linear and channel-independent, so the channel reduction
    commutes with it:  shift = ema(s) with s_t = sum_c w_shift[c] x[c,t],
    and scale = ema(q) with q_t = u_t - 2 shift_t v_t + shift_t^2 E,
    u = sum e_c x^2, v = sum e_c x, E = sum e_c.
  * w_proj is folded into x on the host (xt = w_proj * x, bf16), so
    out = xt*inv - (w_proj*si - b_proj)  with  si = shift*inv,
    inv = 1/sqrt(scale+eps).  Stat weights are divided by w_proj (w_proj^2
    for the square stat) to compensate.
  * All I/O is bf16 (tolerance is 2e-2); stats/scans run in f32.

Implementation notes:
  * Stats are bf16 PE matmuls quadrant-packed at PSUM base partitions
    {0,32,64} so one ACT copy drains 3 groups (engine cost scales with
    free size only).
  * The per-(32-block) scan carries are computed with small PE matmuls
    (lower-triangular r^128-power matrix), avoiding per-scan DMAs.
  * inv/si rows are replicated to 128 partitions with a 7-step SBUF->SBUF
    DMA doubling cascade, keeping the phase-3 DVE ops all-bf16-SBUF
    (tensor_tensor 2x mode, tensor_scalar 4x mode).
"""

import sys
import os

for _p in ("/opt/trn_rl_repo",):
    if _p not in sys.path:
        sys.path.insert(0, _p)

import numpy as np
import ml_dtypes
from contextlib import ExitStack

import concourse.bass as bass
import concourse.bacc as bacc
import concourse.tile as tile
from concourse import mybir
from concourse import bass_utils
from concourse import library_config

MOMENTUM = 0.01
EPS = 1e-6
B, C, T_FULL = 8, 256, 16384
N_CORES = 8
BS = 128          # scan block size (columns per scan block)
NSEG = 4          # pipeline segments over T

F32 = mybir.dt.float32
BF16 = mybir.dt.bfloat16
AOP = mybir.AluOpType
ACTF = mybir.ActivationFunctionType
BF = ml_dtypes.bfloat16


def _host_constants(w_shift, w_scale_log, w_proj, b_proj, T):
    m = MOMENTUM
    r = 1.0 - m
    SEG = T // NSEG
    NB = SEG // BS
    r128 = r ** BS

    ws = w_shift.astype(np.float64)
    e = np.exp(w_scale_log.astype(np.float64))
    wp = w_proj.astype(np.float64)
    bp = b_proj.astype(np.float64)

    # Stat weights, folded for xt = wp*x, zero-padded to 32 output columns
    # (so each quadrant matmul initializes a full 32-partition stripe).
    # The u matmul accumulates into the same PSUM stripe as s/v with its
    # stat at column 2, so one stripe ends up holding [s, v, u, 0...].
    w_sv = np.zeros((128, 2, 32), np.float64)
    w_u = np.zeros((128, 2, 32), np.float64)
    for h in (0, 1):
        sl = slice(128 * h, 128 * (h + 1))
        w_sv[:, h, 0] = m * ws[sl] / wp[sl]
        w_sv[:, h, 1] = -2.0 * m * e[sl] / wp[sl]
        w_u[:, h, 2] = m * e[sl] / wp[sl] ** 2

    # per-half per-partition scalars for the D tensor_scalar
    wpb = np.zeros((128, 2, 2), np.float64)
    for h in (0, 1):
        sl = slice(128 * h, 128 * (h + 1))
        wpb[:, h, 0] = wp[sl]
        wpb[:, h, 1] = bp[sl]

    ecolm = np.full((NB, 1), m * e.sum(), np.float64)

    # scan-carry matmul constants; carry column has 33 output partitions:
    # P_j at partition j<NB, and the next segment's carry-in (P_NB = the
    # end-of-segment value) always at partition 32, so no separate extract
    # matmul is needed and the cin copy stays on one partition.
    mcarry = np.zeros((NB, 33), np.float64)   # lhsT[k, j] = r128^(j-1-k), k<j
    rpow = np.zeros((1, 33), np.float64)
    for j in range(33):
        jj = NB if j == 32 else (j if j < NB else None)
        if jj is None:
            continue
        for k in range(jj):
            mcarry[k, j] = r128 ** (jj - 1 - k)
        rpow[0, j] = r128 ** jj

    f = lambda a: np.ascontiguousarray(a, dtype=np.float32)
    bf = lambda a: np.ascontiguousarray(a.astype(np.float32), dtype=BF)
    return dict(
        w_sv=bf(w_sv), w_u=bf(w_u), wpb=f(wpb), e_col=f(ecolm),
        mcarry=f(mcarry), rpow=f(rpow),
    )


def build_model(T=T_FULL):
    m = MOMENTUM
    r = 1.0 - m
    SEG = T // NSEG
    NB = SEG // BS
    NGS = SEG // 512          # 512-column stat groups per segment

    nc = bacc.Bacc("TRN2", target_bir_lowering=False, debug=False)

    x_d = nc.dram_tensor("x", [128, 2, T], BF16, kind="ExternalInput")
    wsv_d = nc.dram_tensor("w_sv", [128, 2, 32], BF16, kind="ExternalInput")
    wu_d = nc.dram_tensor("w_u", [128, 2, 32], BF16, kind="ExternalInput")
    wpb_d = nc.dram_tensor("wpb", [128, 2, 2], F32, kind="ExternalInput")
    ecol_d = nc.dram_tensor("e_col", [NB, 1], F32, kind="ExternalInput")
    mcar_d = nc.dram_tensor("mcarry", [NB, 33], F32, kind="ExternalInput")
    rpow_d = nc.dram_tensor("rpow", [1, 33], F32, kind="ExternalInput")
    out_d = nc.dram_tensor("out", [128, 2, T], BF16, kind="ExternalOutput")

    with tile.TileContext(nc) as tc, ExitStack() as ctx:
        nc.gpsimd.load_library(library_config.attn)
        consts = ctx.enter_context(tc.tile_pool(name="consts", bufs=1))
        xpool = ctx.enter_context(tc.tile_pool(name="x", bufs=2))
        sqpool = ctx.enter_context(tc.tile_pool(name="sq", bufs=2))
        stpool = ctx.enter_context(tc.tile_pool(name="stage", bufs=2))
        scanp = ctx.enter_context(tc.tile_pool(name="scan", bufs=2))
        bcpool = ctx.enter_context(tc.tile_pool(name="bc", bufs=3))
        zpool = ctx.enter_context(tc.tile_pool(name="z", bufs=2))
        dpool = ctx.enter_context(tc.tile_pool(name="d", bufs=2))
        opool = ctx.enter_context(tc.tile_pool(name="o", bufs=2))
        ps_stat = ctx.enter_context(tc.tile_pool(name="pstat", bufs=2,
                                                 space="PSUM"))
        ps_car = ctx.enter_context(tc.tile_pool(name="pcar", bufs=2,
                                                space="PSUM"))

        # ---- constants ----
        wsv_sb = consts.tile([128, 2, 32], BF16)
        wu_sb = consts.tile([128, 2, 32], BF16)
        wpb_sb = consts.tile([128, 2, 2], F32)
        ecol_sb = consts.tile([NB, 1], F32)
        mcar_sb = consts.tile([NB, 33], F32)
        # rpow and cin live at partition 32 so the carry matmul's second
        # accumulating term (rpow^T @ cin) shares lhsT/rhs base partitions.
        rpow_sb = consts.tile([33, 33], F32)
        for sb, d in ((wsv_sb, wsv_d), (wu_sb, wu_d), (wpb_sb, wpb_d),
                      (ecol_sb, ecol_d), (mcar_sb, mcar_d)):
            nc.sync.dma_start(sb[:], d[:])
        nc.sync.dma_start(rpow_sb[32:33, :], rpow_d[:])
        rfill = consts.tile([NB, BS], F32)
        nc.vector.memset(rfill[:], r)
        eps_sb = consts.tile([NB, 1], F32)
        nc.vector.memset(eps_sb[:], EPS)
        cin = {"shift": consts.tile([33, 1], F32, name="cin_shift"),
               "scale": consts.tile([33, 1], F32, name="cin_scale")}
        nc.vector.memset(cin["shift"][:], 0.0)
        nc.vector.memset(cin["scale"][:], 0.0)

        def ema_scan(src, sig, ccol4, ccol_i):
            """Blockwise EMA scan of src [NB, BS] with PE-matmul carries.
            Carry-column partition 32 is the next segment's carry-in; it is
            copied (same partition) into cin[sig]."""
            loc = scanp.tile([NB, BS], F32, tag=f"{sig}_loc")
            nc.vector.tensor_tensor_scan(
                loc[:], rfill[:], src[:], 0.0, AOP.mult, AOP.add)
            # carry into block j: P_j = sum_{k<j} r128^{j-1-k} y_k + r128^j cin
            nc.tensor.matmul(ccol4[0:33, ccol_i:ccol_i + 1],
                             lhsT=mcar_sb[:], rhs=loc[:, BS - 1:BS],
                             start=True, stop=False)
            nc.tensor.matmul(ccol4[0:33, ccol_i:ccol_i + 1],
                             lhsT=rpow_sb[32:33, :],
                             rhs=cin[sig][32:33, :],
                             start=False, stop=True)
            nc.vector.tensor_copy(cin[sig][32:33, :],
                                  ccol4[32:33, ccol_i:ccol_i + 1])
            fixed = scanp.tile([NB, BS], F32, tag=f"{sig}_fix")
            nc.vector.tensor_tensor_scan(
                fixed[:], rfill[:], src[:], ccol4[0:NB, ccol_i:ccol_i + 1],
                AOP.mult, AOP.add)
            return fixed

        GW = 1024                 # stat group width (columns)
        NG = SEG // GW            # stat groups per segment
        NBG = GW // BS            # scan blocks per group

        for s in range(NSEG):
            seg0 = s * SEG

            # ---- x load (one DMA per segment, sync queue = big I/O only) ----
            xs = xpool.tile([128, 2, SEG], BF16, tag="x")
            nc.sync.dma_start(xs[:], x_d[:, :, seg0:seg0 + SEG])

            # ---- phase 1: stats; groups of GW cols, 3 per PSUM tile at
            # quadrant bases {0,32,64}; u accumulates into the same stripe ----
            S_s = scanp.tile([NB, BS], F32, tag="S_s")
            S_v = scanp.tile([NB, BS], F32, tag="S_v")
            S_u = scanp.tile([NB, BS], F32, tag="S_u")
            g0 = 0
            while g0 < NG:
                ng = min(3, NG - g0)
                pp = 32 * ng
                svu_ps = ps_stat.tile([96, GW], F32, tag="svu")
                for gi in range(ng):
                    g = g0 + gi
                    cols = slice(GW * g, GW * (g + 1))
                    qb = 32 * gi
                    sq = sqpool.tile([128, 2, GW], BF16, tag="sq")
                    for h in (0, 1):
                        nc.scalar.activation(sq[:, h, :], xs[:, h, cols],
                                             ACTF.Square)
                    for j in (0, 1):
                        jsl = slice(512 * j, 512 * (j + 1))
                        jc = slice(cols.start + 512 * j,
                                   cols.start + 512 * (j + 1))
                        nc.tensor.matmul(svu_ps[qb:qb + 32, jsl],
                                         lhsT=wsv_sb[:, 0, :],
                                         rhs=xs[:, 0, jc],
                                         start=True, stop=False)
                        nc.tensor.matmul(svu_ps[qb:qb + 32, jsl],
                                         lhsT=wsv_sb[:, 1, :],
                                         rhs=xs[:, 1, jc],
                                         start=False, stop=False)
                        nc.tensor.matmul(svu_ps[qb:qb + 32, jsl],
                                         lhsT=wu_sb[:, 0, :],
                                         rhs=sq[:, 0, jsl],
                                         start=False, stop=False)
                        nc.tensor.matmul(svu_ps[qb:qb + 32, jsl],
                                         lhsT=wu_sb[:, 1, :],
                                         rhs=sq[:, 1, jsl],
                                         start=False, stop=True)
                stage = stpool.tile([96, GW], F32, tag="stg")
                nc.scalar.copy(stage[0:pp, :], svu_ps[0:pp, :])
                # scatter stat rows to scan-block layout (gpsimd SWDGE queue)
                for gi in range(ng):
                    g = g0 + gi
                    bsl = slice(NBG * g, NBG * (g + 1))
                    nc.gpsimd.dma_start(S_s[bsl, :],
                                        stage[32 * gi + 0:32 * gi + 1, :])
                    nc.gpsimd.dma_start(S_v[bsl, :],
                                        stage[32 * gi + 1:32 * gi + 2, :])
                    nc.gpsimd.dma_start(S_u[bsl, :],
                                        stage[32 * gi + 2:32 * gi + 3, :])
                g0 += ng

            # ---- phase 2: scans ----
            ccol4 = ps_car.tile([33, 2], F32, tag="ccol")
            shift_S = ema_scan(S_s, "shift", ccol4, 0)

            t1 = scanp.tile([NB, BS], F32, tag="t1")
            nc.vector.tensor_tensor(t1[:], shift_S[:], S_v[:], AOP.mult)
            t2 = scanp.tile([NB, BS], F32, tag="t2")
            nc.vector.tensor_tensor(t2[:], shift_S[:], shift_S[:], AOP.mult)
            q1 = scanp.tile([NB, BS], F32, tag="q1")
            nc.vector.scalar_tensor_tensor(
                q1[:], t2[:], ecol_sb[:], S_u[:], AOP.mult, AOP.add)
            qm = scanp.tile([NB, BS], F32, tag="qm")
            nc.vector.tensor_tensor(qm[:], q1[:], t1[:], AOP.add)

            scale_S = ema_scan(qm, "scale", ccol4, 1)

            sq_s = scanp.tile([NB, BS], F32, tag="sq_s")
            nc.scalar.activation(sq_s[:], scale_S[:], ACTF.Sqrt,
                                 bias=eps_sb[:])
            inv_S = scanp.tile([NB, BS], F32, tag="inv_S")
            nc.vector.reciprocal_approx_fast(inv_S[:], sq_s[:])
            sib = scanp.tile([NB, BS], BF16, tag="sib")
            nc.vector.tensor_tensor(sib[:], shift_S[:], inv_S[:], AOP.mult)
            invb = scanp.tile([NB, BS], BF16, tag="invb")
            nc.vector.tensor_copy(invb[:], inv_S[:])

            # ---- broadcast rows to 128 partitions (doubling cascade) ----
            bc = bcpool.tile([128, 2 * SEG], BF16, tag="bc")
            nc.scalar.dma_start(bc[0:1, 0:SEG], invb[:])
            nc.scalar.dma_start(bc[0:1, SEG:2 * SEG], sib[:])
            # inv half: DMA doubling cascade; si half: gpsimd broadcast.
            # The two paths run on different resources in parallel.
            nc.gpsimd.partition_broadcast(bc[:, SEG:2 * SEG],
                                          bc[0:1, SEG:2 * SEG])
            k = 1
            while k < 128:
                nc.scalar.dma_start(bc[k:2 * k, 0:SEG], bc[0:k, 0:SEG])
                k *= 2

            # ---- phase 3: out = xt*inv - (wp*si - bp) ----
            bcinv = bass.AP(bc[:, 0:SEG].tensor, bc[:, 0:SEG].offset,
                            [list(bc[:, 0:SEG].ap[0]), [0, 2],
                             list(bc[:, 0:SEG].ap[1])])
            z = zpool.tile([128, 2, SEG], BF16, tag="z")
            nc.vector.tensor_tensor(z[:], xs[:], bcinv, AOP.mult)
            o = opool.tile([128, 2, SEG], BF16, tag="o")
            for h in (0, 1):
                D = dpool.tile([128, SEG], BF16, tag="D")
                nc.vector.tensor_scalar(D[:], bc[:, SEG:2 * SEG],
                                        wpb_sb[:, h, 0:1], wpb_sb[:, h, 1:2],
                                        AOP.mult, AOP.subtract)
                nc.vector.tensor_tensor(o[:, h, :], z[:, h, :], D[:],
                                        AOP.subtract)
            nc.sync.dma_start(out_d[:, :, seg0:seg0 + SEG], o[:])

    nc.compile()
    return nc


_MODEL_CACHE = {}


def _get_model(T=T_FULL):
    if T not in _MODEL_CACHE:
        _MODEL_CACHE[T] = build_model(T)
    return _MODEL_CACHE[T]


def make_in_maps(x, w_shift, w_scale_log, w_proj, b_proj, T):
    """Per-core input dicts (core i gets batch i)."""
    consts = _host_constants(w_shift, w_scale_log, w_proj, b_proj, T)
    nb = x.shape[0]
    wp = w_proj.astype(np.float32)
    in_maps = []
    for i in range(nb):
        xt = (x[i].astype(np.float32) * wp[:, None]).astype(BF)
        xt = np.ascontiguousarray(
            np.stack([xt[:128], xt[128:]], axis=1))      # [128, 2, T]
        im = {"x": xt}
        im.update(consts)
        in_maps.append(im)
    return in_maps


def kernel(x, w_shift, w_scale_log, w_proj, b_proj):
    T = x.shape[-1]
    nc = _get_model(T)
    in_maps = make_in_maps(x, w_shift, w_scale_log, w_proj, b_proj, T)
    res = bass_utils.run_bass_kernel_spmd(
        nc, in_maps, core_ids=list(range(len(in_maps))))
    outs = []
    for i in range(len(in_maps)):
        o = np.asarray(res.results[i]["out"])            # [128, 2, T] bf16
        outs.append(np.concatenate([o[:, 0, :], o[:, 1, :]], axis=0))
    return np.stack(outs, 0).astype(np.float32)
